# revision 44
# baseline (speedup 1.0000x reference)
"""Block-sparse attention (CABAttention) Trainium2 kernel, v2.

Sharding: 8 cores = 2 batches x 4 head-groups (2 head-pairs each).
Per core, all fp16 datapath (fp32 PSUM/denominators):
 - qkv projection: fp16 matmuls, 8 K-chunk accumulation, N=512 tiles
 - top-2+diag block-sparse attention; block selection on host in
   float64 (matches fp32 reference ordering), consumed as dynamic
   access-pattern offsets on matmul moving operands; one 8-register
   PE load serves the 4 offsets of two consecutive blocks
 - head-B scores/AV run in quadrant (0,64) against base-0 replicas
   (qkB = [qT-B | kkT-B], vdAB = [vd-A | vd-B]) because dynamic APs
   must sit at partition base 0 (HW erratum); a single offset value
   (64j for A, N+64j for B) serves both the scores and AV operands
 - exp via one scalar activation with accum_out denominator (dup-diag
   masked by a -30000 bias added beforehand); normalization applied
   AFTER the AV matmul on [128,64] (cheaper than on [128,192] probs)
 - probs transposed per slot with one full-height [128,64]->[64,128]
   PE transpose (both heads at base 0); output transposed back with a
   (0,0)/(64,64) quadrant pair
 - pair-1 qkv/vdup and the output projection are woven between
   attention blocks (phase overlap; their long N=512 streams also
   raise PE duty against HAM clock throttling)
 - output projection row-parallel: fp16 partial y per core, summed on
   host in fp32 (+bias).

Known HW errata found while building this (avoid): dynamic-offset APs
(matmul rhs or DMA src) silently read the wrong data at partition
base 64; two PE transposes in row-disjoint quadrants sharing a column
group fail; a (64,64) static matmul mixed between (0,0)/(0,64)
dynamic matmuls fails; gpsimd software-DGE DMA wedges the device.
"""
import sys

sys.path.insert(0, "/opt/trn_rl_repo")

import numpy as np

import concourse.bass as bass
import concourse.mybir as mybir
import concourse.tile as tile
from concourse import bacc
from concourse.bass import ds
from concourse.bass_utils import run_bass_kernel_spmd
from concourse.masks import make_identity

F32 = mybir.dt.float32
F16 = mybir.dt.float16
I32 = mybir.dt.int32

DIM = 1024
H = 16
HD = 64
BS = 64
N = 2048
B = 2
M = N // BS            # 32 blocks
SCALE = HD ** -0.5
NCORES = 8
HPC = H // (NCORES // B)   # 4 heads per core

_NC_CACHE = None
LAST_RESULTS = None


def build_kernel(stage=5, sub=3):
    nc = bacc.Bacc(None)
    xt_d = nc.dram_tensor("xt", [DIM, N], F16, kind="ExternalInput")
    wq_d = nc.dram_tensor("wq", [DIM, 768], F16, kind="ExternalInput")
    pw_d = nc.dram_tensor("pw", [256, DIM], F16, kind="ExternalInput")
    idx_d = nc.dram_tensor("selidx", [1, 256], I32, kind="ExternalInput")
    wb_d = nc.dram_tensor("wbias", [128, 64], F32, kind="ExternalInput")
    y_d = nc.dram_tensor("y", [N, DIM], F16, kind="ExternalOutput")

    EXP = mybir.ActivationFunctionType.Exp
    ADD = mybir.AluOpType.add
    MUL = mybir.AluOpType.mult

    with tile.TileContext(nc) as tc:
        with tc.tile_pool(name="big", bufs=1) as big, \
             tc.tile_pool(name="wrk", bufs=4) as wrk:

            # ---- persistent SBUF tensors ----
            xts = big.tile([128, 8, N], F16)
            wqs = big.tile([128, 8, 768], F16)
            pwt = big.tile([128, 2, DIM], F16)
            idx = big.tile([1, 256], I32)
            wb = big.tile([128, 64], F32)
            qT = [big.tile([128, N], F16, name=f"qT{i}") for i in range(2)]
            kkT = [big.tile([128, N], F16, name=f"kkT{i}") for i in range(2)]
            vvT = [big.tile([128, N], F16, name=f"vvT{i}") for i in range(2)]
            # base-0 replicas for head B: [qT-B | kkT-B] so one dynamic
            # offset value (N + 64j) indexes the kk region
            qkB = [big.tile([64, 2 * N], F16, name=f"qkB{i}") for i in range(2)]
            # [vd-A | vd-B] keys-major v, same offset convention
            vdAB = [big.tile([64, 2 * N], F16, name=f"vdAB{i}") for i in range(2)]
            outT = [big.tile([128, N], F16, name=f"outT{i}") for i in range(2)]
            identf = big.tile([128, 128], F32)
            ident = big.tile([128, 128], F16)

            # ---- input DMAs (split for pipelining) ----
            xt_v = xt_d[:].rearrange("(a p) n -> p a n", p=128)
            wq_v = wq_d[:].rearrange("(a p) n -> p a n", p=128)
            pw_v = pw_d[:].rearrange("(a p) n -> p a n", p=128)
            for k in range(8):
                nc.sync.dma_start(xts[:, k, :], xt_v[:, k, :])
                nc.sync.dma_start(wqs[:, k, :], wq_v[:, k, :])
            nc.sync.dma_start(pwt[:], pw_v[:])
            nc.sync.dma_start(idx[:], idx_d[:])
            nc.sync.dma_start(wb[:], wb_d[:])

            make_identity(nc, identf[:])
            nc.vector.tensor_copy(ident[:], identf[:])
            if stage < 5:
                for t_ in outT:
                    nc.gpsimd.memset(t_[:], 0.0)

            def copy_out(eng, dst, src):
                if eng is nc.scalar:
                    eng.copy(dst, src)
                else:
                    eng.tensor_copy(dst, src)

            # ---- qkv projection: fp16, 8 K-chunk accumulation ----
            # mt -> target; pair-0 tiles first so attention p0 can start
            tgt = {0: qT[0], 1: qT[1], 2: kkT[0], 3: vvT[0],
                   4: kkT[1], 5: vvT[1]}
            ci = 0

            def emit_qkv_group(pool, mt, nt):
                nonlocal ci
                ps = pool.tile([128, 512], F32, name="qk_ps", tag="aux")
                for k in range(8):
                    nc.tensor.matmul(
                        ps[:],
                        lhsT=wqs[:, k, mt * 128:(mt + 1) * 128],
                        rhs=xts[:, k, nt * 512:(nt + 1) * 512],
                        start=(k == 0), stop=(k == 7))
                eng = (nc.vector, nc.scalar)[ci % 2]
                ci += 1
                copy_out(eng, tgt[mt][:, nt * 512:(nt + 1) * 512], ps[:])

            def emit_vdup_j(pool, p, j):
                nonlocal ci
                tp = pool.tile([64, 128], F16, name="vd_tp", tag="aux")
                nc.tensor.transpose(
                    tp[:], vvT[p][:, j * 64:(j + 1) * 64], ident[:])
                # one strided copy: A block -> cols [j*64], B block
                # -> cols [N + j*64] of vdAB
                dst = vdAB[p][:].rearrange(
                    "q (g c) -> q g c", g=2)[:, :, j * 64:(j + 1) * 64]
                src = tp[:].rearrange("q (g c) -> q g c", g=2)
                eng = (nc.vector, nc.scalar)[ci % 2]
                ci += 1
                copy_out(eng, dst, src)

            # phase A: pair-0 qkv + vdup in dedicated pools (closed after)
            with tc.tile_pool(name="qkps", bufs=4, space="PSUM") as qkps, \
                 tc.tile_pool(name="vtps", bufs=2, space="PSUM") as vtps:
                # warm-up: solid matmul streams during the input-DMA wait
                # so the PE clock (HAM) is at 8/8 when real work arrives
                warmsrc = big.tile([128, 512], F16)
                nc.gpsimd.memset(warmsrc[:], 0.5)
                wps = qkps.tile([128, 512], F32, name="wps", tag="aux")
                for _ in range(24):
                    nc.tensor.matmul(wps[:], lhsT=ident[:],
                                     rhs=warmsrc[:], start=True, stop=True)
                for mt in (3, 0, 2):
                    for nt in range(4):
                        emit_qkv_group(qkps, mt, nt)
                    if mt == 3:
                        # vdup-p0 right after vvT[0]; its transposes and
                        # copies overlap the remaining qkv groups
                        for j in range(M):
                            emit_vdup_j(vtps, 0, j)
                nc.sync.dma_start(qkB[0][:, 0:N], qT[0][64:128, :])
                nc.sync.dma_start(qkB[0][:, N:2 * N], kkT[0][64:128, :])

            # ---- block-sparse attention; qkv-p1 / vdup-p1 / proj
            # matmuls woven between blocks (their long N=512 streams keep
            # the HAM activity monitor from throttling the PE clock) ----
            with tc.tile_pool(name="spsp", bufs=2, space="PSUM") as spsp, \
                 tc.tile_pool(name="ptps", bufs=2, space="PSUM") as ptps, \
                 tc.tile_pool(name="avps", bufs=2, space="PSUM") as avps, \
                 tc.tile_pool(name="otps", bufs=1, space="PSUM") as otps, \
                 tc.tile_pool(name="auxp", bufs=1, space="PSUM") as auxp:

                def emit_proj(tt):
                    ts_ = slice(tt * 128, (tt + 1) * 128)
                    for nt in range(2):
                        ns = slice(nt * 512, (nt + 1) * 512)
                        yp = auxp.tile([128, 512], F32, name="yp",
                                       tag="aux")
                        nc.tensor.matmul(yp[:], lhsT=outT[0][:, ts_],
                                         rhs=pwt[:, 0, ns],
                                         start=True, stop=False)
                        nc.tensor.matmul(yp[:], lhsT=outT[1][:, ts_],
                                         rhs=pwt[:, 1, ns],
                                         start=False, stop=True)
                        ys = wrk.tile([128, 512], F16, tag="ys")
                        copy_out((nc.vector, nc.scalar)[tt % 2], ys[:],
                                 yp[:])
                        nc.sync.dma_start(y_d[ts_, ns], ys[:])

                def weave(ph, qb):
                    # real big-stream PE work between attention blocks
                    if ph == 0:
                        if qb < 12:
                            mt = (1, 4, 5)[qb // 4]
                            emit_qkv_group(auxp, mt, qb % 4)
                            if qb == 7:
                                nc.sync.dma_start(qkB[1][:, 0:N],
                                                  qT[1][64:128, :])
                                nc.sync.dma_start(qkB[1][:, N:2 * N],
                                                  kkT[1][64:128, :])
                        elif qb < 20:
                            for j in range(4 * (qb - 12), 4 * (qb - 11)):
                                emit_vdup_j(auxp, 1, j)
                    else:
                        if qb % 2 == 1:
                            emit_proj(qb // 2)

                anchors = []
                t = -1
                for p in (range(2) if stage >= 2 else []):
                    for qb in range(M):
                        t += 1
                        base = p * 128 + qb * 4
                        col = p * 32 + qb
                        qs = slice(qb * 64, (qb + 1) * 64)

                        # 4 dynamic offsets per block; one 16-register PE
                        # load serves four consecutive blocks
                        eng = nc.tensor
                        LB = 2   # blocks per load
                        if t % LB == 0:
                            regs = [eng.alloc_register(f"off_{base + c}")
                                    for c in range(4 * LB)]
                            li = eng.reg_load(regs,
                                              idx[0:1, base:base + 4 * LB])
                            if t >= 3 * LB:
                                tile.add_dep_helper(
                                    li.ins, anchors[t - 3 * LB].ins,
                                    sync=False,
                                    reason="bound PE register live range")
                            pend_offs = []
                            for c in range(4 * LB):
                                lo = 0 if c % 4 < 2 else N
                                pend_offs.append(eng.snap(
                                    regs[c], donate=True, min_val=lo,
                                    max_val=lo + N - 64))
                        offs = pend_offs[(t % LB) * 4:(t % LB) * 4 + 4]
                        soA = [offs[0], offs[1], qb * 64]
                        soB = [offs[2], offs[3], N + qb * 64]

                        # same stationary operand for all 3 slots of a
                        # head -> emit consecutively
                        sps = spsp.tile([128, 192], F32)
                        for s in range(3):
                            cs = slice(s * 64, (s + 1) * 64)
                            nc.tensor.matmul(
                                sps[0:64, cs], lhsT=qT[p][0:64, qs],
                                rhs=kkT[p][0:64, ds(soA[s], 64)],
                                start=True, stop=True)
                        for s in range(3):
                            cs = slice(s * 64, (s + 1) * 64)
                            nc.tensor.matmul(
                                sps[64:128, cs], lhsT=qkB[p][:, qs],
                                rhs=qkB[p][:, ds(soB[s], 64)],
                                start=True, stop=True,
                                skip_group_check=True,
                                tile_position=(0, 64))

                        # duplicate-diag mask, then one exp with accum den
                        nc.vector.tensor_scalar(
                            sps[:, 128:192], sps[:, 128:192],
                            wb[:, col:col + 1], None, op0=ADD)
                        pu = wrk.tile([128, 192], F16, tag="pu")
                        den = wrk.tile([128, 1], F32, tag="den")
                        nc.scalar.activation(pu[:], sps[:], EXP,
                                             accum_out=den[:])

                        if stage < 3:
                            anchors.append(li)
                            continue
                        # transpose probs per (head, slot): [64,64] pairs
                        # in disjoint quadrants run concurrently
                        # full-height transpose per slot: [128 q, 64 k]
                        # -> [64 k, 128 (qA|qB)], both heads at base 0
                        pt = ptps.tile([64, 384], F16)
                        for s in range(3):
                            cs = slice(s * 64, (s + 1) * 64)
                            nc.tensor.transpose(
                                pt[0:64, s * 128:(s + 1) * 128],
                                pu[:, cs], ident[:])
                        pts = wrk.tile([64, 384], F16, tag="pts")
                        nc.vector.tensor_copy(pts[:], pt[:])

                        if stage < 4:
                            anchors.append(li)
                            continue
                        # AV (unnormalized): A -> av[0:64], B -> av[64:128]
                        av = avps.tile([128, 64], F32)
                        for s in range(3):
                            nc.tensor.matmul(
                                av[0:64, :],
                                lhsT=pts[0:64, s * 128:s * 128 + 64],
                                rhs=vdAB[p][:, ds(soA[s], 64)],
                                start=(s == 0), stop=(s == 2))
                            mi = nc.tensor.matmul(
                                av[64:128, :],
                                lhsT=pts[0:64, s * 128 + 64:(s + 1) * 128],
                                rhs=vdAB[p][:, ds(soB[s], 64)],
                                start=(s == 0), stop=(s == 2),
                                skip_group_check=True,
                                tile_position=(0, 64))
                        anchors.append(mi)

                        # normalize post-AV (64 cols instead of 192)
                        rden = wrk.tile([128, 1], F32, tag="rden")
                        nc.vector.reciprocal(rden[:], den[:])
                        o = wrk.tile([128, 64], F16, tag="o")
                        nc.vector.tensor_scalar(o[:], av[:], rden[:, 0:1],
                                                None, op0=MUL)

                        if stage < 5:
                            continue
                        # transpose back to [hd, q]; A/B quadrant pair
                        otp = otps.tile([128, 64], F16)
                        nc.tensor.transpose(otp[0:64, :], o[0:64, :],
                                            ident[0:64, 0:64])
                        nc.tensor.transpose(otp[64:128, :], o[64:128, :],
                                            ident[64:128, 64:128],
                                            tile_position=(64, 64))
                        nc.scalar.copy(outT[p][:, qs], otp[:])

                        if stage >= 5:
                            weave(p, qb)

    nc.finalize()
    return nc


def _host_prep(x, qkv_w, proj_w):
    """Per-core input maps + block selection (float64, matches fp32 ref)."""
    in_maps = []
    x64 = x.astype(np.float64)
    for core in range(NCORES):
        b = core // (NCORES // B)
        hg = core % (NCORES // B)
        heads = [hg * HPC + i for i in range(HPC)]

        xt = np.ascontiguousarray(x[b].T).astype(np.float16)

        wqkvT = np.empty((DIM, 768), np.float32)
        for p in range(2):
            hA, hB = heads[2 * p], heads[2 * p + 1]
            wqkvT[:, p*128:p*128+64] = qkv_w[hA*64:(hA+1)*64].T * SCALE
            wqkvT[:, p*128+64:p*128+128] = qkv_w[hB*64:(hB+1)*64].T * SCALE
            kbase = 256 + p * 256
            wqkvT[:, kbase:kbase+64] = qkv_w[DIM+hA*64:DIM+(hA+1)*64].T
            wqkvT[:, kbase+64:kbase+128] = qkv_w[DIM+hB*64:DIM+(hB+1)*64].T
            vbase = kbase + 128
            wqkvT[:, vbase:vbase+64] = qkv_w[2*DIM+hA*64:2*DIM+(hA+1)*64].T
            wqkvT[:, vbase+64:vbase+128] = qkv_w[2*DIM+hB*64:2*DIM+(hB+1)*64].T

        pw = np.ascontiguousarray(
            proj_w[:, heads[0]*64:(heads[-1]+1)*64].T).astype(np.float16)

        # float64 selection (matches fp32 reference ordering w/ margin)
        xb = x64[b].reshape(M, BS, DIM).mean(axis=1)
        selidx = np.zeros((1, 256), np.int32)
        wbias = np.zeros((128, 64), np.float32)
        for p in range(2):
            for hip in range(2):
                h = heads[2 * p + hip]
                qb_ = xb @ qkv_w[h*64:(h+1)*64].T.astype(np.float64)
                kb_ = xb @ qkv_w[DIM+h*64:DIM+(h+1)*64].T.astype(np.float64)
                c = qb_ @ kb_.T
                for i in range(M):
                    order = np.argsort(-c[i], kind="stable")
                    i1, i2 = int(order[0]), int(order[1])
                    col = p * 128 + i * 4 + hip * 2
                    selidx[0, col] = hip * N + i1 * 64
                    selidx[0, col + 1] = hip * N + i2 * 64
                    if i == i1 or i == i2:
                        wbias[hip*64:(hip+1)*64, p*32+i] = -30000.0
        in_maps.append({"xt": xt, "wq": wqkvT.astype(np.float16),
                        "pw": pw, "selidx": selidx, "wbias": wbias})
    return in_maps


def kernel(x, qkv_w, proj_w, proj_b):
    global _NC_CACHE, LAST_RESULTS
    x = np.asarray(x, np.float32)
    qkv_w = np.asarray(qkv_w, np.float32)
    proj_w = np.asarray(proj_w, np.float32)
    proj_b = np.asarray(proj_b, np.float32)

    if _NC_CACHE is None:
        _NC_CACHE = build_kernel()
    nc = _NC_CACHE

    in_maps = _host_prep(x, qkv_w, proj_w)
    res = run_bass_kernel_spmd(nc, in_maps, list(range(NCORES)))
    LAST_RESULTS = res

    out = np.zeros((B, N, DIM), np.float32)
    for core in range(NCORES):
        out[core // (NCORES // B)] += res.results[core]["y"].astype(np.float32)
    out += proj_b[None, None, :]
    return out
